# revision 36
# baseline (speedup 1.0000x reference)
"""AtomicConvLayer (GNN message passing) on 8 Trainium2 NeuronCores.

Reference computation (per atom i, neighbors j = nbr[i, 0..31]):
    h_ij   = relu(x_i @ W1a + x_j @ W1b + b1)         (msg_W1 split in two)
    agg_i  = sum_j (h_ij @ W2 + b2)
    u_i    = relu(x_i @ U1a + agg_i @ U1b + bu1)
    out_i  = relu(x_i + u_i @ UW2 + bu2)

Algebraic restructuring used here (exact in exact arithmetic):
    B      = X @ W1b                (25600x128 table, computed per core)
    A_i    = x_i @ W1a + b1
    Hsum_i = sum_j relu(A_i + B[nbr_ij])             <- only gather B rows
    u_i    = relu(x_i @ U1a + Hsum_i @ (W2 @ U1b) + (bu1 + 32*b2 @ U1b))
    out_i  = relu(x_i + u_i @ UW2 + bu2)

Sharding: data-parallel over atoms. Each core owns 3200 consecutive atoms
(25000 padded to 25600), holds the full atom table, computes the full B
table locally (12.8 MB, cheaper than cross-core gathers), then gathers its
own 3200*32 neighbor rows from B with dma_gather.
"""

import sys

sys.path.insert(0, "/opt/trn_rl_repo")

import numpy as np

N_ATOMS = 25000
N_PAD = 25600          # 8 cores x 3200
D = 128
M = 32                 # neighbors per atom
N_CORES = 8
OWN = N_PAD // N_CORES          # 3200 atoms per core
BLOCKS = OWN // 128             # 25 blocks of 128 atoms per core
TILES = N_PAD // 128            # 200 tiles in the full table
LOAD_CHUNK = 16                 # tiles per phase-1 B write

_CACHE = {}
last_results = None


def _build_nc():
    import concourse.bacc as bacc
    import concourse.mybir as mybir
    import concourse.tile as tile
    from concourse.bass_interp import get_hw_module
    from concourse.masks import make_identity

    f32 = mybir.dt.float32
    bf16 = mybir.dt.bfloat16
    nc = bacc.Bacc("TRN2", target_bir_lowering=False, debug=False,
                   num_swdge_queues=4)

    atoms16_d = nc.dram_tensor("atoms16", [N_PAD, D], mybir.dt.bfloat16,
                               kind="ExternalInput")
    ownx_d = nc.dram_tensor("own_x", [OWN, D], f32, kind="ExternalInput")
    ownx16_d = nc.dram_tensor("own_x16", [OWN, D], mybir.dt.bfloat16,
                              kind="ExternalInput")
    idx_d = nc.dram_tensor("idx16", [128, BLOCKS * 256], mybir.dt.int16, kind="ExternalInput")
    w1a_d = nc.dram_tensor("w1a", [D, D], f32, kind="ExternalInput")
    w1b_d = nc.dram_tensor("w1b", [D, D], f32, kind="ExternalInput")
    b1_d = nc.dram_tensor("b1", [1, D], f32, kind="ExternalInput")
    w2_d = nc.dram_tensor("w2", [D, D], f32, kind="ExternalInput")
    b2c_d = nc.dram_tensor("b2c", [D, 1], f32, kind="ExternalInput")
    u1a_d = nc.dram_tensor("u1a", [D, D], f32, kind="ExternalInput")
    u1b_d = nc.dram_tensor("u1b", [D, D], f32, kind="ExternalInput")
    bu1_d = nc.dram_tensor("bu1", [1, D], f32, kind="ExternalInput")
    uw2_d = nc.dram_tensor("uw2", [D, D], f32, kind="ExternalInput")
    bu2_d = nc.dram_tensor("bu2", [1, D], f32, kind="ExternalInput")
    out_d = nc.dram_tensor("out", [OWN, D], f32, kind="ExternalOutput")

    out_v = out_d.rearrange("(n p) d -> p n d", p=128)       # [128, 25, 128]

    with tile.TileContext(nc) as tc:
        with (
            tc.tile_pool(name="persist", bufs=1) as per,
            tc.tile_pool(name="dram", bufs=1, space="DRAM") as dram,
        ):
            ident = per.tile([128, 128], f32)
            make_identity(nc, ident[:])
            ident16 = per.tile([128, 128], bf16)
            nc.vector.tensor_copy(ident16[:], ident[:])
            ones_row = per.tile([1, 128], f32)
            nc.gpsimd.memset(ones_row[:], 1.0)

            w1a = per.tile([D, D], f32)
            w1b = per.tile([D, D], f32)
            b1 = per.tile([1, D], f32)
            w2 = per.tile([D, D], f32)
            b2c = per.tile([D, 1], f32)
            u1a = per.tile([D, D], f32)
            u1b = per.tile([D, D], f32)
            bu1 = per.tile([1, D], f32)
            uw2 = per.tile([D, D], f32)
            bu2 = per.tile([1, D], f32)
            idx_sb = per.tile([128, BLOCKS * 256], mybir.dt.int16)
            for sb, d in [(w1a, w1a_d), (w1b, w1b_d), (b1, b1_d), (w2, w2_d),
                          (b2c, b2c_d), (u1a, u1a_d), (u1b, u1b_d), (bu1, bu1_d),
                          (uw2, uw2_d), (bu2, bu2_d), (idx_sb, idx_d)]:
                nc.sync.dma_start(sb[:], d[:])

            w1a16 = per.tile([D, D], bf16)
            w1b16 = per.tile([D, D], bf16)
            u1a16 = per.tile([D, D], bf16)
            uw216 = per.tile([D, D], bf16)
            nc.vector.tensor_copy(w1a16[:], w1a[:])
            nc.vector.tensor_copy(w1b16[:], w1b[:])
            nc.vector.tensor_copy(u1a16[:], u1a[:])
            nc.vector.tensor_copy(uw216[:], uw2[:])

            x_own = per.tile([128, BLOCKS, D], f32)
            xT_own = per.tile([128, OWN], bf16)
            a_own = per.tile([128, BLOCKS, D], bf16)
            ostage = per.tile([128, BLOCKS, D], f32)
            w2u = per.tile([D, D], f32)
            biasu = per.tile([1, D], f32)

            bdram = dram.tile([N_PAD, D], bf16)
            bdram_v = bdram[:].rearrange("(n p) d -> p n d", p=128)

            # ---- weight folds: w2u = W2 @ U1b ; biasu = bu1 + 32*b2 @ U1b
            with tc.tile_pool(name="ps0", bufs=1, space="PSUM") as ps0:
                ps_wt = ps0.tile([128, 128], f32)
                nc.tensor.transpose(ps_wt[:], w2[:], ident[:])
                w2t = per.tile([D, D], f32)
                nc.vector.tensor_copy(w2t[:], ps_wt[:])
                ps_w2u = ps0.tile([128, 128], f32)
                nc.tensor.matmul(ps_w2u[:], w2t[:], u1b[:], start=True, stop=True)
                nc.vector.tensor_copy(w2u[:], ps_w2u[:])
                w2u16 = per.tile([D, D], bf16)
                nc.vector.tensor_copy(w2u16[:], ps_w2u[:])

                b1rep = per.tile([128, D], f32)
                ps_b1 = ps0.tile([128, 128], f32, tag="ps_b1")
                nc.tensor.matmul(ps_b1[:], ones_row[:], b1[:], start=True, stop=True)
                nc.vector.tensor_copy(b1rep[:], ps_b1[:])

                b2s = per.tile([D, 1], f32)
                nc.vector.tensor_scalar_mul(b2s[:], b2c[:], float(M))
                ps_c = ps0.tile([1, 128], f32)
                nc.tensor.matmul(ps_c[:], b2s[:], u1b[:], start=True, stop=True)
                nc.vector.tensor_tensor(out=biasu[:], in0=ps_c[:], in1=bu1[:],
                                        op=mybir.AluOpType.add)

            # ---- phase 1: B = atoms @ W1b  -> bdram
            # atoms16 is loaded pre-transposed via the DMA xbar, so the
            # matmul lhsT ([feat_in, atoms]) comes straight from DRAM.
            with tc.tile_pool(name="p1", bufs=2) as p1, \
                 tc.tile_pool(name="ps1", bufs=2, space="PSUM") as ps1:
                # all xbar-transposed loads up-front (one xbar window; a
                # mode transition mid-stream serializes the DMA engines)
                xtT = p1.tile([128, N_PAD], bf16, tag="xtT", bufs=1)
                pieces = [6400, 6400, 6400, 6400]
                r0 = 0
                for pl in pieces:
                    nc.sync.dma_start_transpose(
                        xtT[:, r0:r0 + pl], atoms16_d[r0:r0 + pl, :])
                    r0 += pl
                nc.sync.dma_start_transpose(xT_own[:], ownx16_d[:])
                ownx_v = ownx_d.rearrange("(n p) d -> p n d", p=128)
                nc.sync.dma_start(x_own[:], ownx_v[:])

                t0 = 0
                while t0 < TILES:
                    k = min(LOAD_CHUNK, TILES - t0)
                    bstage = p1.tile([128, LOAD_CHUNK, D], bf16, tag="bstage", bufs=6)
                    # 4 B-tile matmuls share one PSUM bank -> one wide copy
                    for i0 in range(0, k, 4):
                        ps_b = ps1.tile([128, 512], f32, tag="ps_b")
                        for i in range(i0, min(i0 + 4, k)):
                            t = t0 + i
                            nc.tensor.matmul(
                                ps_b[:, (i - i0) * D:(i - i0 + 1) * D],
                                xtT[:, t * D:(t + 1) * D],
                                w1b16[:], start=True, stop=True)
                        kk = min(i0 + 4, k) - i0
                        nc.vector.tensor_copy(
                            bstage[:, i0:i0 + kk, :].rearrange("p a b -> p (a b)"),
                            ps_b[:, :kk * D])
                    nc.scalar.dma_start(bdram_v[:, t0:t0 + k, :], bstage[:, :k, :])
                    t0 += k

                # ---- phase 1b: A = x@W1a+b1
                for b in range(BLOCKS):
                    ps_a = ps1.tile([128, 128], f32, tag="ps_b")
                    nc.tensor.matmul(ps_a[:], xT_own[:, b * D:(b + 1) * D],
                                     w1a16[:], start=True, stop=True)
                    nc.vector.tensor_tensor(
                        out=a_own[:, b, :], in0=ps_a[:],
                        in1=b1rep[:],
                        op=mybir.AluOpType.add)

            # ---- phase 2+3: gather, Hsum, update net
            with tc.tile_pool(name="p2", bufs=6) as p2, \
                 tc.tile_pool(name="psh", bufs=2, space="PSUM") as psh, \
                 tc.tile_pool(name="ps2t", bufs=1, space="PSUM") as ps2t, \
                 tc.tile_pool(name="ps2", bufs=2, space="PSUM") as ps2:
                def emit_gather(b):
                    g = p2.tile([128, M, D], bf16, tag="g")
                    half = M * 128 // 2
                    nc.gpsimd.dma_gather(
                        g[:, :M // 2, :], bdram[:],
                        idx_sb[:, b * 256:b * 256 + 128],
                        half, half, D, single_packet=False,
                        queue_num=(2 * b) % 4,
                    )
                    nc.gpsimd.dma_gather(
                        g[:, M // 2:, :], bdram[:],
                        idx_sb[:, b * 256 + 128:(b + 1) * 256],
                        half, half, D, single_packet=False,
                        queue_num=(2 * b + 1) % 4,
                    )
                    return g

                def emit_addrelu(b, g):
                    for h in (0, 1):
                        gh = g[:, h * (M // 2):(h + 1) * (M // 2), :]
                        nc.vector.tensor_tensor(
                            out=gh, in0=gh,
                            in1=a_own[:, b:b + 1, :].to_broadcast(
                                [128, M // 2, D]),
                            op=mybir.AluOpType.add,
                        )
                        nc.scalar.activation(gh, gh,
                                             mybir.ActivationFunctionType.Relu)

                def emit_finish(b, g):
                    # Hsum split: slots 16..31 accumulate on PE, 0..15 on DVE
                    ps_h = psh.tile([128, 128], f32, tag="ps_h")
                    for m in range(M // 2, M):
                        nc.tensor.matmul(ps_h[:], ident16[:], g[:, m, :],
                                         start=(m == M // 2), stop=(m == M - 1))
                    hs = p2.tile([128, 128], f32, tag="hs")
                    nc.vector.reduce_sum(
                        out=hs[:], in_=g[:, :M // 2, :].rearrange("p m f -> p f m"),
                        axis=mybir.AxisListType.X)
                    nc.vector.tensor_tensor(out=hs[:], in0=hs[:], in1=ps_h[:],
                                            op=mybir.AluOpType.add)

                    ps_ht = ps2t.tile([128, 128], f32, tag="ps_ht")
                    nc.tensor.transpose(ps_ht[:], hs[:], ident[:])
                    hst = p2.tile([128, 128], bf16, tag="hst")
                    nc.scalar.copy(hst[:], ps_ht[:])

                    ps_pre = ps2.tile([128, 128], f32, tag="ps_pre")
                    nc.tensor.matmul(ps_pre[:], xT_own[:, b * D:(b + 1) * D], u1a16[:], start=True, stop=False)
                    nc.tensor.matmul(ps_pre[:], hst[:], w2u16[:], start=False, stop=False)
                    nc.tensor.matmul(ps_pre[:], ones_row[:], biasu[:], start=False, stop=True)
                    u = p2.tile([128, 128], bf16, tag="u")
                    nc.scalar.activation(u[:], ps_pre[:],
                                         mybir.ActivationFunctionType.Relu)

                    ps_ut = ps2t.tile([128, 128], bf16, tag="ps_ut")
                    nc.tensor.transpose(ps_ut[:], u[:], ident16[:])
                    ut = p2.tile([128, 128], bf16, tag="ut")
                    nc.scalar.copy(ut[:], ps_ut[:])

                    ps_o = ps2.tile([128, 128], f32, tag="ps_o")
                    nc.tensor.matmul(ps_o[:], ut[:], uw216[:], start=True, stop=False)
                    nc.tensor.matmul(ps_o[:], ones_row[:], bu2[:], start=False, stop=False)
                    nc.tensor.matmul(ps_o[:], ident[:], x_own[:, b, :], start=False, stop=True)
                    nc.scalar.activation(ostage[:, b, :], ps_o[:],
                                         mybir.ActivationFunctionType.Relu)

                # software pipeline: gather 2 ahead, add/relu 1 ahead
                gs = {0: emit_gather(0)}
                if BLOCKS > 1:
                    gs[1] = emit_gather(1)
                emit_addrelu(0, gs[0])
                for b in range(BLOCKS):
                    if b + 2 < BLOCKS:
                        gs[b + 2] = emit_gather(b + 2)
                    if b + 1 < BLOCKS:
                        emit_addrelu(b + 1, gs[b + 1])
                    emit_finish(b, gs.pop(b))
                    if b % 5 == 4:
                        nc.sync.dma_start(out_v[:, b - 4:b + 1, :],
                                          ostage[:, b - 4:b + 1, :])


    nc.compile()
    nc.m = get_hw_module(nc.m)
    return nc


def get_nc():
    if "nc" not in _CACHE:
        _CACHE["nc"] = _build_nc()
    return _CACHE["nc"]


def make_in_maps(atom_features, nbr_indices,
                 msg_W1, msg_b1, msg_W2, msg_b2,
                 upd_W1, upd_b1, upd_W2, upd_b2):
    atom_features = np.ascontiguousarray(np.asarray(atom_features, dtype=np.float32))
    nbr = np.asarray(nbr_indices)

    atoms = np.zeros((N_PAD, D), dtype=np.float32)
    atoms[:N_ATOMS] = atom_features
    import ml_dtypes
    atoms16 = atoms.astype(ml_dtypes.bfloat16)

    idx = np.zeros((N_PAD, M), dtype=np.int16)
    idx[:N_ATOMS] = nbr.astype(np.int16)
    # per core/block: logical order j = m*128 + p; wrapped [16, 256] then
    # replicated to 128 partitions: unwrapped[j] = tile[j % 16, j // 16]
    idx = idx.reshape(N_CORES, BLOCKS, 128, M)
    idx = idx.transpose(0, 1, 3, 2)                 # [core, blk, m, p] -> L[j]
    idx = idx.reshape(N_CORES, BLOCKS * M * 128 // 16, 16)
    idx = idx.transpose(0, 2, 1)                    # [core, 16, 6400]
    idx16 = np.tile(idx, (1, 8, 1))                 # [core, 128, 6400]
    idx16 = np.ascontiguousarray(idx16)

    w = {
        "w1a": np.ascontiguousarray(np.asarray(msg_W1[:D], dtype=np.float32)),
        "w1b": np.ascontiguousarray(np.asarray(msg_W1[D:], dtype=np.float32)),
        "b1": np.asarray(msg_b1, dtype=np.float32).reshape(1, D),
        "w2": np.ascontiguousarray(np.asarray(msg_W2, dtype=np.float32)),
        "b2c": np.asarray(msg_b2, dtype=np.float32).reshape(D, 1),
        "u1a": np.ascontiguousarray(np.asarray(upd_W1[:D], dtype=np.float32)),
        "u1b": np.ascontiguousarray(np.asarray(upd_W1[D:], dtype=np.float32)),
        "bu1": np.asarray(upd_b1, dtype=np.float32).reshape(1, D),
        "uw2": np.ascontiguousarray(np.asarray(upd_W2, dtype=np.float32)),
        "bu2": np.asarray(upd_b2, dtype=np.float32).reshape(1, D),
    }

    in_maps = []
    for c in range(N_CORES):
        m = {
            "atoms16": atoms16,
            "own_x": atoms[c * OWN:(c + 1) * OWN],
            "own_x16": atoms16[c * OWN:(c + 1) * OWN],
            "idx16": idx16[c],
        }
        m.update(w)
        in_maps.append(m)
    return in_maps


def kernel(atom_features, nbr_features, nbr_indices,
           msg_W1, msg_b1, msg_W2, msg_b2,
           upd_W1, upd_b1, upd_W2, upd_b2):
    global last_results
    from concourse.bass_utils import run_bass_kernel_spmd

    nc = get_nc()
    in_maps = make_in_maps(atom_features, nbr_indices,
                           msg_W1, msg_b1, msg_W2, msg_b2,
                           upd_W1, upd_b1, upd_W2, upd_b2)
    res = run_bass_kernel_spmd(nc, in_maps, core_ids=list(range(N_CORES)))
    last_results = res
    out = np.concatenate([res.results[c]["out"] for c in range(N_CORES)], axis=0)
    return out[:N_ATOMS]


# revision 38
# speedup vs baseline: 1.0293x; 1.0293x over previous
"""AtomicConvLayer (GNN message passing) on 8 Trainium2 NeuronCores.

Reference computation (per atom i, neighbors j = nbr[i, 0..31]):
    h_ij   = relu(x_i @ W1a + x_j @ W1b + b1)         (msg_W1 split in two)
    agg_i  = sum_j (h_ij @ W2 + b2)
    u_i    = relu(x_i @ U1a + agg_i @ U1b + bu1)
    out_i  = relu(x_i + u_i @ UW2 + bu2)

Algebraic restructuring used here (exact in exact arithmetic):
    B      = X @ W1b                (25600x128 table, computed per core)
    A_i    = x_i @ W1a + b1
    Hsum_i = sum_j relu(A_i + B[nbr_ij])             <- only gather B rows
    u_i    = relu(x_i @ U1a + Hsum_i @ (W2 @ U1b) + (bu1 + 32*b2 @ U1b))
    out_i  = relu(x_i + u_i @ UW2 + bu2)

Sharding: data-parallel over atoms. Each core owns 3200 consecutive atoms
(25000 padded to 25600), holds the full atom table, computes the full B
table locally (12.8 MB, cheaper than cross-core gathers), then gathers its
own 3200*32 neighbor rows from B with dma_gather.
"""

import sys

sys.path.insert(0, "/opt/trn_rl_repo")

import numpy as np

N_ATOMS = 25000
N_PAD = 25600          # 8 cores x 3200
D = 128
M = 32                 # neighbors per atom
N_CORES = 8
OWN = N_PAD // N_CORES          # 3200 atoms per core
BLOCKS = OWN // 128             # 25 blocks of 128 atoms per core
TILES = N_PAD // 128            # 200 tiles in the full table
LOAD_CHUNK = 16                 # tiles per phase-1 B write

_CACHE = {}
last_results = None


def _build_nc():
    import concourse.bacc as bacc
    import concourse.mybir as mybir
    import concourse.tile as tile
    from concourse.bass_interp import get_hw_module
    from concourse.masks import make_identity

    f32 = mybir.dt.float32
    bf16 = mybir.dt.bfloat16
    nc = bacc.Bacc("TRN2", target_bir_lowering=False, debug=False,
                   num_swdge_queues=4)

    atoms16_d = nc.dram_tensor("atoms16", [N_PAD, D], mybir.dt.bfloat16,
                               kind="ExternalInput")
    ownx_d = nc.dram_tensor("own_x", [OWN, D], f32, kind="ExternalInput")
    ownx16_d = nc.dram_tensor("own_x16", [OWN, D], mybir.dt.bfloat16,
                              kind="ExternalInput")
    idx_d = nc.dram_tensor("idx16", [128, BLOCKS * 256], mybir.dt.int16, kind="ExternalInput")
    w1a_d = nc.dram_tensor("w1a", [D, D], f32, kind="ExternalInput")
    w1b_d = nc.dram_tensor("w1b", [D, D], f32, kind="ExternalInput")
    b1_d = nc.dram_tensor("b1", [1, D], f32, kind="ExternalInput")
    w2_d = nc.dram_tensor("w2", [D, D], f32, kind="ExternalInput")
    b2c_d = nc.dram_tensor("b2c", [D, 1], f32, kind="ExternalInput")
    u1a_d = nc.dram_tensor("u1a", [D, D], f32, kind="ExternalInput")
    u1b_d = nc.dram_tensor("u1b", [D, D], f32, kind="ExternalInput")
    bu1_d = nc.dram_tensor("bu1", [1, D], f32, kind="ExternalInput")
    uw2_d = nc.dram_tensor("uw2", [D, D], f32, kind="ExternalInput")
    bu2_d = nc.dram_tensor("bu2", [1, D], f32, kind="ExternalInput")
    out_d = nc.dram_tensor("out", [OWN, D], f32, kind="ExternalOutput")

    out_v = out_d.rearrange("(n p) d -> p n d", p=128)       # [128, 25, 128]

    with tile.TileContext(nc) as tc:
        with (
            tc.tile_pool(name="persist", bufs=1) as per,
            tc.tile_pool(name="dram", bufs=1, space="DRAM") as dram,
        ):
            ident = per.tile([128, 128], f32)
            make_identity(nc, ident[:])
            ident16 = per.tile([128, 128], bf16)
            nc.vector.tensor_copy(ident16[:], ident[:])
            ones_row = per.tile([1, 128], f32)
            nc.gpsimd.memset(ones_row[:], 1.0)

            w1a = per.tile([D, D], f32)
            w1b = per.tile([D, D], f32)
            b1 = per.tile([1, D], f32)
            w2 = per.tile([D, D], f32)
            b2c = per.tile([D, 1], f32)
            u1a = per.tile([D, D], f32)
            u1b = per.tile([D, D], f32)
            bu1 = per.tile([1, D], f32)
            uw2 = per.tile([D, D], f32)
            bu2 = per.tile([1, D], f32)
            idx_sb = per.tile([128, BLOCKS * 256], mybir.dt.int16)
            for sb, d in [(w1a, w1a_d), (w1b, w1b_d), (b1, b1_d), (w2, w2_d),
                          (b2c, b2c_d), (u1a, u1a_d), (u1b, u1b_d), (bu1, bu1_d),
                          (uw2, uw2_d), (bu2, bu2_d), (idx_sb, idx_d)]:
                nc.sync.dma_start(sb[:], d[:])

            w1a16 = per.tile([D, D], bf16)
            w1b16 = per.tile([D, D], bf16)
            u1a16 = per.tile([D, D], bf16)
            uw216 = per.tile([D, D], bf16)
            nc.vector.tensor_copy(w1a16[:], w1a[:])
            nc.vector.tensor_copy(w1b16[:], w1b[:])
            nc.vector.tensor_copy(u1a16[:], u1a[:])
            nc.vector.tensor_copy(uw216[:], uw2[:])

            x_own = per.tile([128, BLOCKS, D], f32)
            xT_own = per.tile([128, OWN], bf16)
            a_own = per.tile([128, BLOCKS, D], bf16)
            ostage = per.tile([128, BLOCKS, D], f32)
            w2u = per.tile([D, D], f32)
            biasu = per.tile([1, D], f32)

            bdram = dram.tile([N_PAD, D], bf16)
            bdram_v = bdram[:].rearrange("(n p) d -> p n d", p=128)

            # ---- weight folds: w2u = W2 @ U1b ; biasu = bu1 + 32*b2 @ U1b
            with tc.tile_pool(name="ps0", bufs=1, space="PSUM") as ps0:
                ps_wt = ps0.tile([128, 128], f32)
                nc.tensor.transpose(ps_wt[:], w2[:], ident[:])
                w2t = per.tile([D, D], f32)
                nc.vector.tensor_copy(w2t[:], ps_wt[:])
                ps_w2u = ps0.tile([128, 128], f32)
                nc.tensor.matmul(ps_w2u[:], w2t[:], u1b[:], start=True, stop=True)
                nc.vector.tensor_copy(w2u[:], ps_w2u[:])
                w2u16 = per.tile([D, D], bf16)
                nc.vector.tensor_copy(w2u16[:], ps_w2u[:])

                b1rep = per.tile([128, D], f32)
                ps_b1 = ps0.tile([128, 128], f32, tag="ps_b1")
                nc.tensor.matmul(ps_b1[:], ones_row[:], b1[:], start=True, stop=True)
                nc.vector.tensor_copy(b1rep[:], ps_b1[:])

                b2s = per.tile([D, 1], f32)
                nc.vector.tensor_scalar_mul(b2s[:], b2c[:], float(M))
                ps_c = ps0.tile([1, 128], f32)
                nc.tensor.matmul(ps_c[:], b2s[:], u1b[:], start=True, stop=True)
                nc.vector.tensor_tensor(out=biasu[:], in0=ps_c[:], in1=bu1[:],
                                        op=mybir.AluOpType.add)

            # ---- phase 1: B = atoms @ W1b  -> bdram
            # atoms16 is loaded pre-transposed via the DMA xbar, so the
            # matmul lhsT ([feat_in, atoms]) comes straight from DRAM.
            with tc.tile_pool(name="p1", bufs=2) as p1, \
                 tc.tile_pool(name="ps1", bufs=2, space="PSUM") as ps1:
                # all xbar-transposed loads up-front (one xbar window; a
                # mode transition mid-stream serializes the DMA engines)
                xtT = p1.tile([128, N_PAD], bf16, tag="xtT", bufs=1)
                pieces = [6400, 6400, 6400, 6400]
                r0 = 0
                for pl in pieces:
                    nc.sync.dma_start_transpose(
                        xtT[:, r0:r0 + pl], atoms16_d[r0:r0 + pl, :])
                    r0 += pl
                nc.sync.dma_start_transpose(xT_own[:], ownx16_d[:])
                ownx_v = ownx_d.rearrange("(n p) d -> p n d", p=128)
                nc.sync.dma_start(x_own[:], ownx_v[:])

                t0 = 0
                while t0 < TILES:
                    k = min(LOAD_CHUNK, TILES - t0)
                    bstage = p1.tile([128, LOAD_CHUNK, D], bf16, tag="bstage", bufs=8)
                    # 4 B-tile matmuls share one PSUM bank -> one wide copy
                    for i0 in range(0, k, 4):
                        ps_b = ps1.tile([128, 512], f32, tag="ps_b")
                        for i in range(i0, min(i0 + 4, k)):
                            t = t0 + i
                            nc.tensor.matmul(
                                ps_b[:, (i - i0) * D:(i - i0 + 1) * D],
                                xtT[:, t * D:(t + 1) * D],
                                w1b16[:], start=True, stop=True)
                        kk = min(i0 + 4, k) - i0
                        nc.vector.tensor_copy(
                            bstage[:, i0:i0 + kk, :].rearrange("p a b -> p (a b)"),
                            ps_b[:, :kk * D])
                    nc.sync.dma_start(bdram_v[:, t0:t0 + k, :], bstage[:, :k, :])
                    t0 += k

                # ---- phase 1b: A = x@W1a+b1
                for b in range(BLOCKS):
                    ps_a = ps1.tile([128, 128], f32, tag="ps_b")
                    nc.tensor.matmul(ps_a[:], xT_own[:, b * D:(b + 1) * D],
                                     w1a16[:], start=True, stop=True)
                    nc.vector.tensor_tensor(
                        out=a_own[:, b, :], in0=ps_a[:],
                        in1=b1rep[:],
                        op=mybir.AluOpType.add)

            # ---- phase 2+3: gather, Hsum, update net
            with tc.tile_pool(name="p2", bufs=6) as p2, \
                 tc.tile_pool(name="psh", bufs=2, space="PSUM") as psh, \
                 tc.tile_pool(name="ps2t", bufs=1, space="PSUM") as ps2t, \
                 tc.tile_pool(name="ps2", bufs=2, space="PSUM") as ps2:
                def emit_gather(b):
                    g = p2.tile([128, M, D], bf16, tag="g")
                    half = M * 128 // 2
                    nc.gpsimd.dma_gather(
                        g[:, :M // 2, :], bdram[:],
                        idx_sb[:, b * 256:b * 256 + 128],
                        half, half, D, single_packet=False,
                        queue_num=(2 * b) % 4,
                    )
                    nc.gpsimd.dma_gather(
                        g[:, M // 2:, :], bdram[:],
                        idx_sb[:, b * 256 + 128:(b + 1) * 256],
                        half, half, D, single_packet=False,
                        queue_num=(2 * b + 1) % 4,
                    )
                    return g

                def emit_addrelu(b, g):
                    for h in (0, 1):
                        gh = g[:, h * (M // 2):(h + 1) * (M // 2), :]
                        nc.vector.tensor_tensor(
                            out=gh, in0=gh,
                            in1=a_own[:, b:b + 1, :].to_broadcast(
                                [128, M // 2, D]),
                            op=mybir.AluOpType.add,
                        )
                        nc.scalar.activation(gh, gh,
                                             mybir.ActivationFunctionType.Relu)

                def emit_finish(b, g):
                    # Hsum split: slots 16..31 accumulate on PE, 0..15 on DVE
                    ps_h = psh.tile([128, 128], f32, tag="ps_h")
                    for m in range(M // 2, M):
                        nc.tensor.matmul(ps_h[:], ident16[:], g[:, m, :],
                                         start=(m == M // 2), stop=(m == M - 1))
                    hs = p2.tile([128, 128], f32, tag="hs")
                    nc.vector.reduce_sum(
                        out=hs[:], in_=g[:, :M // 2, :].rearrange("p m f -> p f m"),
                        axis=mybir.AxisListType.X)
                    nc.vector.tensor_tensor(out=hs[:], in0=hs[:], in1=ps_h[:],
                                            op=mybir.AluOpType.add)

                    ps_ht = ps2t.tile([128, 128], f32, tag="ps_ht")
                    nc.tensor.transpose(ps_ht[:], hs[:], ident[:])
                    hst = p2.tile([128, 128], bf16, tag="hst")
                    nc.scalar.copy(hst[:], ps_ht[:])

                    ps_pre = ps2.tile([128, 128], f32, tag="ps_pre")
                    nc.tensor.matmul(ps_pre[:], xT_own[:, b * D:(b + 1) * D], u1a16[:], start=True, stop=False)
                    nc.tensor.matmul(ps_pre[:], hst[:], w2u16[:], start=False, stop=False)
                    nc.tensor.matmul(ps_pre[:], ones_row[:], biasu[:], start=False, stop=True)
                    u = p2.tile([128, 128], bf16, tag="u")
                    nc.scalar.activation(u[:], ps_pre[:],
                                         mybir.ActivationFunctionType.Relu)

                    ps_ut = ps2t.tile([128, 128], bf16, tag="ps_ut")
                    nc.tensor.transpose(ps_ut[:], u[:], ident16[:])
                    ut = p2.tile([128, 128], bf16, tag="ut")
                    nc.scalar.copy(ut[:], ps_ut[:])

                    ps_o = ps2.tile([128, 128], f32, tag="ps_o")
                    nc.tensor.matmul(ps_o[:], ut[:], uw216[:], start=True, stop=False)
                    nc.tensor.matmul(ps_o[:], ones_row[:], bu2[:], start=False, stop=False)
                    nc.tensor.matmul(ps_o[:], ident[:], x_own[:, b, :], start=False, stop=True)
                    nc.scalar.activation(ostage[:, b, :], ps_o[:],
                                         mybir.ActivationFunctionType.Relu)

                # software pipeline: gather 2 ahead, add/relu 1 ahead
                gs = {0: emit_gather(0)}
                if BLOCKS > 1:
                    gs[1] = emit_gather(1)
                emit_addrelu(0, gs[0])
                for b in range(BLOCKS):
                    if b + 2 < BLOCKS:
                        gs[b + 2] = emit_gather(b + 2)
                    if b + 1 < BLOCKS:
                        emit_addrelu(b + 1, gs[b + 1])
                    emit_finish(b, gs.pop(b))
                    if b % 5 == 4:
                        nc.sync.dma_start(out_v[:, b - 4:b + 1, :],
                                          ostage[:, b - 4:b + 1, :])


    nc.compile()
    nc.m = get_hw_module(nc.m)
    return nc


def get_nc():
    if "nc" not in _CACHE:
        _CACHE["nc"] = _build_nc()
    return _CACHE["nc"]


def make_in_maps(atom_features, nbr_indices,
                 msg_W1, msg_b1, msg_W2, msg_b2,
                 upd_W1, upd_b1, upd_W2, upd_b2):
    atom_features = np.ascontiguousarray(np.asarray(atom_features, dtype=np.float32))
    nbr = np.asarray(nbr_indices)

    atoms = np.zeros((N_PAD, D), dtype=np.float32)
    atoms[:N_ATOMS] = atom_features
    import ml_dtypes
    atoms16 = atoms.astype(ml_dtypes.bfloat16)

    idx = np.zeros((N_PAD, M), dtype=np.int16)
    idx[:N_ATOMS] = nbr.astype(np.int16)
    # per core/block: logical order j = m*128 + p; wrapped [16, 256] then
    # replicated to 128 partitions: unwrapped[j] = tile[j % 16, j // 16]
    idx = idx.reshape(N_CORES, BLOCKS, 128, M)
    idx = idx.transpose(0, 1, 3, 2)                 # [core, blk, m, p] -> L[j]
    idx = idx.reshape(N_CORES, BLOCKS * M * 128 // 16, 16)
    idx = idx.transpose(0, 2, 1)                    # [core, 16, 6400]
    idx16 = np.tile(idx, (1, 8, 1))                 # [core, 128, 6400]
    idx16 = np.ascontiguousarray(idx16)

    w = {
        "w1a": np.ascontiguousarray(np.asarray(msg_W1[:D], dtype=np.float32)),
        "w1b": np.ascontiguousarray(np.asarray(msg_W1[D:], dtype=np.float32)),
        "b1": np.asarray(msg_b1, dtype=np.float32).reshape(1, D),
        "w2": np.ascontiguousarray(np.asarray(msg_W2, dtype=np.float32)),
        "b2c": np.asarray(msg_b2, dtype=np.float32).reshape(D, 1),
        "u1a": np.ascontiguousarray(np.asarray(upd_W1[:D], dtype=np.float32)),
        "u1b": np.ascontiguousarray(np.asarray(upd_W1[D:], dtype=np.float32)),
        "bu1": np.asarray(upd_b1, dtype=np.float32).reshape(1, D),
        "uw2": np.ascontiguousarray(np.asarray(upd_W2, dtype=np.float32)),
        "bu2": np.asarray(upd_b2, dtype=np.float32).reshape(1, D),
    }

    in_maps = []
    for c in range(N_CORES):
        m = {
            "atoms16": atoms16,
            "own_x": atoms[c * OWN:(c + 1) * OWN],
            "own_x16": atoms16[c * OWN:(c + 1) * OWN],
            "idx16": idx16[c],
        }
        m.update(w)
        in_maps.append(m)
    return in_maps


def kernel(atom_features, nbr_features, nbr_indices,
           msg_W1, msg_b1, msg_W2, msg_b2,
           upd_W1, upd_b1, upd_W2, upd_b2):
    global last_results
    from concourse.bass_utils import run_bass_kernel_spmd

    nc = get_nc()
    in_maps = make_in_maps(atom_features, nbr_indices,
                           msg_W1, msg_b1, msg_W2, msg_b2,
                           upd_W1, upd_b1, upd_W2, upd_b2)
    res = run_bass_kernel_spmd(nc, in_maps, core_ids=list(range(N_CORES)))
    last_results = res
    out = np.concatenate([res.results[c]["out"] for c in range(N_CORES)], axis=0)
    return out[:N_ATOMS]


# revision 39
# speedup vs baseline: 1.0374x; 1.0079x over previous
"""AtomicConvLayer (GNN message passing) on 8 Trainium2 NeuronCores.

Reference computation (per atom i, neighbors j = nbr[i, 0..31]):
    h_ij   = relu(x_i @ W1a + x_j @ W1b + b1)         (msg_W1 split in two)
    agg_i  = sum_j (h_ij @ W2 + b2)
    u_i    = relu(x_i @ U1a + agg_i @ U1b + bu1)
    out_i  = relu(x_i + u_i @ UW2 + bu2)

Algebraic restructuring used here (exact in exact arithmetic):
    B      = X @ W1b                (25600x128 table, computed per core)
    A_i    = x_i @ W1a + b1
    Hsum_i = sum_j relu(A_i + B[nbr_ij])             <- only gather B rows
    u_i    = relu(x_i @ U1a + Hsum_i @ (W2 @ U1b) + (bu1 + 32*b2 @ U1b))
    out_i  = relu(x_i + u_i @ UW2 + bu2)

Sharding: data-parallel over atoms. Each core owns 3200 consecutive atoms
(25000 padded to 25600), holds the full atom table, computes the full B
table locally (12.8 MB, cheaper than cross-core gathers), then gathers its
own 3200*32 neighbor rows from B with dma_gather.
"""

import sys

sys.path.insert(0, "/opt/trn_rl_repo")

import numpy as np

N_ATOMS = 25000
N_PAD = 25600          # 8 cores x 3200
D = 128
M = 32                 # neighbors per atom
N_CORES = 8
OWN = N_PAD // N_CORES          # 3200 atoms per core
BLOCKS = OWN // 128             # 25 blocks of 128 atoms per core
TILES = N_PAD // 128            # 200 tiles in the full table
LOAD_CHUNK = 16                 # tiles per phase-1 B write

_CACHE = {}
last_results = None


def _build_nc():
    import concourse.bacc as bacc
    import concourse.mybir as mybir
    import concourse.tile as tile
    from concourse.bass_interp import get_hw_module
    from concourse.masks import make_identity

    f32 = mybir.dt.float32
    bf16 = mybir.dt.bfloat16
    nc = bacc.Bacc("TRN2", target_bir_lowering=False, debug=False,
                   num_swdge_queues=4)

    atoms16_d = nc.dram_tensor("atoms16", [N_PAD, D], mybir.dt.bfloat16,
                               kind="ExternalInput")
    ownx_d = nc.dram_tensor("own_x", [OWN, D], f32, kind="ExternalInput")
    ownx16_d = nc.dram_tensor("own_x16", [OWN, D], mybir.dt.bfloat16,
                              kind="ExternalInput")
    idx_d = nc.dram_tensor("idx16", [128, BLOCKS * 256], mybir.dt.int16, kind="ExternalInput")
    w1a_d = nc.dram_tensor("w1a", [D, D], f32, kind="ExternalInput")
    w1b_d = nc.dram_tensor("w1b", [D, D], f32, kind="ExternalInput")
    b1_d = nc.dram_tensor("b1", [1, D], f32, kind="ExternalInput")
    w2_d = nc.dram_tensor("w2", [D, D], f32, kind="ExternalInput")
    b2c_d = nc.dram_tensor("b2c", [D, 1], f32, kind="ExternalInput")
    u1a_d = nc.dram_tensor("u1a", [D, D], f32, kind="ExternalInput")
    u1b_d = nc.dram_tensor("u1b", [D, D], f32, kind="ExternalInput")
    bu1_d = nc.dram_tensor("bu1", [1, D], f32, kind="ExternalInput")
    uw2_d = nc.dram_tensor("uw2", [D, D], f32, kind="ExternalInput")
    bu2_d = nc.dram_tensor("bu2", [1, D], f32, kind="ExternalInput")
    out_d = nc.dram_tensor("out", [OWN, D], f32, kind="ExternalOutput")

    out_v = out_d.rearrange("(n p) d -> p n d", p=128)       # [128, 25, 128]

    with tile.TileContext(nc) as tc:
        with (
            tc.tile_pool(name="persist", bufs=1) as per,
            tc.tile_pool(name="dram", bufs=1, space="DRAM") as dram,
        ):
            ident = per.tile([128, 128], f32)
            make_identity(nc, ident[:])
            ident16 = per.tile([128, 128], bf16)
            nc.vector.tensor_copy(ident16[:], ident[:])
            ones_row = per.tile([1, 128], f32)
            nc.gpsimd.memset(ones_row[:], 1.0)

            w1a = per.tile([D, D], f32)
            w1b = per.tile([D, D], f32)
            b1 = per.tile([1, D], f32)
            w2 = per.tile([D, D], f32)
            b2c = per.tile([D, 1], f32)
            u1a = per.tile([D, D], f32)
            u1b = per.tile([D, D], f32)
            bu1 = per.tile([1, D], f32)
            uw2 = per.tile([D, D], f32)
            bu2 = per.tile([1, D], f32)
            idx_sb = per.tile([128, BLOCKS * 256], mybir.dt.int16)
            for sb, d in [(w1a, w1a_d), (w1b, w1b_d), (b1, b1_d), (w2, w2_d),
                          (b2c, b2c_d), (u1a, u1a_d), (u1b, u1b_d), (bu1, bu1_d),
                          (uw2, uw2_d), (bu2, bu2_d), (idx_sb, idx_d)]:
                nc.sync.dma_start(sb[:], d[:])

            w1a16 = per.tile([D, D], bf16)
            w1b16 = per.tile([D, D], bf16)
            u1a16 = per.tile([D, D], bf16)
            uw216 = per.tile([D, D], bf16)
            nc.vector.tensor_copy(w1a16[:], w1a[:])
            nc.vector.tensor_copy(w1b16[:], w1b[:])
            nc.vector.tensor_copy(u1a16[:], u1a[:])
            nc.vector.tensor_copy(uw216[:], uw2[:])

            x_own = per.tile([128, BLOCKS, D], f32)
            xT_own = per.tile([128, OWN], bf16)
            a_own = per.tile([128, BLOCKS, D], bf16)
            ostage = per.tile([128, BLOCKS, D], f32)
            w2u = per.tile([D, D], f32)
            biasu = per.tile([1, D], f32)

            bdram = dram.tile([N_PAD, D], bf16)
            bdram_v = bdram[:].rearrange("(n p) d -> p n d", p=128)

            # ---- weight folds: w2u = W2 @ U1b ; biasu = bu1 + 32*b2 @ U1b
            with tc.tile_pool(name="ps0", bufs=1, space="PSUM") as ps0:
                ps_wt = ps0.tile([128, 128], f32)
                nc.tensor.transpose(ps_wt[:], w2[:], ident[:])
                w2t = per.tile([D, D], f32)
                nc.vector.tensor_copy(w2t[:], ps_wt[:])
                ps_w2u = ps0.tile([128, 128], f32)
                nc.tensor.matmul(ps_w2u[:], w2t[:], u1b[:], start=True, stop=True)
                nc.vector.tensor_copy(w2u[:], ps_w2u[:])
                w2u16 = per.tile([D, D], bf16)
                nc.vector.tensor_copy(w2u16[:], ps_w2u[:])

                b1rep = per.tile([128, D], f32)
                ps_b1 = ps0.tile([128, 128], f32, tag="ps_b1")
                nc.tensor.matmul(ps_b1[:], ones_row[:], b1[:], start=True, stop=True)
                nc.vector.tensor_copy(b1rep[:], ps_b1[:])

                b2s = per.tile([D, 1], f32)
                nc.vector.tensor_scalar_mul(b2s[:], b2c[:], float(M))
                ps_c = ps0.tile([1, 128], f32)
                nc.tensor.matmul(ps_c[:], b2s[:], u1b[:], start=True, stop=True)
                nc.vector.tensor_tensor(out=biasu[:], in0=ps_c[:], in1=bu1[:],
                                        op=mybir.AluOpType.add)

            # ---- phase 1: B = atoms @ W1b  -> bdram
            # atoms16 is loaded pre-transposed via the DMA xbar, so the
            # matmul lhsT ([feat_in, atoms]) comes straight from DRAM.
            with tc.tile_pool(name="p1", bufs=2) as p1, \
                 tc.tile_pool(name="ps1", bufs=2, space="PSUM") as ps1:
                # all xbar-transposed loads up-front (one xbar window; a
                # mode transition mid-stream serializes the DMA engines)
                xtT = p1.tile([128, N_PAD], bf16, tag="xtT", bufs=1)
                pieces = [6400, 6400, 6400, 6400]
                r0 = 0
                for pl in pieces:
                    nc.sync.dma_start_transpose(
                        xtT[:, r0:r0 + pl], atoms16_d[r0:r0 + pl, :])
                    r0 += pl
                nc.sync.dma_start_transpose(xT_own[:], ownx16_d[:])
                ownx_v = ownx_d.rearrange("(n p) d -> p n d", p=128)
                nc.sync.dma_start(x_own[:], ownx_v[:])

                t0 = 0
                while t0 < TILES:
                    k = min(LOAD_CHUNK, TILES - t0)
                    bstage = p1.tile([128, LOAD_CHUNK, D], bf16, tag="bstage", bufs=6)
                    # 4 B-tile matmuls share one PSUM bank -> one wide copy
                    for i0 in range(0, k, 4):
                        ps_b = ps1.tile([128, 512], f32, tag="ps_b")
                        for i in range(i0, min(i0 + 4, k)):
                            t = t0 + i
                            nc.tensor.matmul(
                                ps_b[:, (i - i0) * D:(i - i0 + 1) * D],
                                xtT[:, t * D:(t + 1) * D],
                                w1b16[:], start=True, stop=True)
                        kk = min(i0 + 4, k) - i0
                        nc.vector.tensor_copy(
                            bstage[:, i0:i0 + kk, :].rearrange("p a b -> p (a b)"),
                            ps_b[:, :kk * D])
                    nc.sync.dma_start(bdram_v[:, t0:t0 + k, :], bstage[:, :k, :])
                    t0 += k

                # ---- phase 1b: A = x@W1a+b1
                for b in range(BLOCKS):
                    ps_a = ps1.tile([128, 128], f32, tag="ps_b")
                    nc.tensor.matmul(ps_a[:], xT_own[:, b * D:(b + 1) * D],
                                     w1a16[:], start=True, stop=True)
                    nc.vector.tensor_tensor(
                        out=a_own[:, b, :], in0=ps_a[:],
                        in1=b1rep[:],
                        op=mybir.AluOpType.add)

            # ---- phase 2+3: gather, Hsum, update net
            with tc.tile_pool(name="p2", bufs=6) as p2, \
                 tc.tile_pool(name="psh", bufs=2, space="PSUM") as psh, \
                 tc.tile_pool(name="ps2t", bufs=1, space="PSUM") as ps2t, \
                 tc.tile_pool(name="ps2", bufs=2, space="PSUM") as ps2:
                def emit_gather(b):
                    g = p2.tile([128, M, D], bf16, tag="g")
                    half = M * 128 // 2
                    nc.gpsimd.dma_gather(
                        g[:, :M // 2, :], bdram[:],
                        idx_sb[:, b * 256:b * 256 + 128],
                        half, half, D, single_packet=False,
                        queue_num=(2 * b) % 4,
                    )
                    nc.gpsimd.dma_gather(
                        g[:, M // 2:, :], bdram[:],
                        idx_sb[:, b * 256 + 128:(b + 1) * 256],
                        half, half, D, single_packet=False,
                        queue_num=(2 * b + 1) % 4,
                    )
                    return g

                def emit_addrelu(b, g):
                    for h in (0, 1):
                        gh = g[:, h * (M // 2):(h + 1) * (M // 2), :]
                        nc.vector.tensor_tensor(
                            out=gh, in0=gh,
                            in1=a_own[:, b:b + 1, :].to_broadcast(
                                [128, M // 2, D]),
                            op=mybir.AluOpType.add,
                        )
                        nc.scalar.activation(gh, gh,
                                             mybir.ActivationFunctionType.Relu)

                def emit_finish(b, g):
                    # Hsum split: slots 16..31 accumulate on PE, 0..15 on DVE
                    ps_h = psh.tile([128, 128], f32, tag="ps_h")
                    for m in range(M // 2, M):
                        nc.tensor.matmul(ps_h[:], ident16[:], g[:, m, :],
                                         start=(m == M // 2), stop=(m == M - 1))
                    hs = p2.tile([128, 128], f32, tag="hs")
                    nc.vector.reduce_sum(
                        out=hs[:], in_=g[:, :M // 2, :].rearrange("p m f -> p f m"),
                        axis=mybir.AxisListType.X)
                    nc.vector.tensor_tensor(out=hs[:], in0=hs[:], in1=ps_h[:],
                                            op=mybir.AluOpType.add)

                    ps_ht = ps2t.tile([128, 128], f32, tag="ps_ht")
                    nc.tensor.transpose(ps_ht[:], hs[:], ident[:])
                    hst = p2.tile([128, 128], bf16, tag="hst")
                    nc.scalar.copy(hst[:], ps_ht[:])

                    ps_pre = ps2.tile([128, 128], f32, tag="ps_pre")
                    nc.tensor.matmul(ps_pre[:], xT_own[:, b * D:(b + 1) * D], u1a16[:], start=True, stop=False)
                    nc.tensor.matmul(ps_pre[:], hst[:], w2u16[:], start=False, stop=False)
                    nc.tensor.matmul(ps_pre[:], ones_row[:], biasu[:], start=False, stop=True)
                    u = p2.tile([128, 128], bf16, tag="u")
                    nc.scalar.activation(u[:], ps_pre[:],
                                         mybir.ActivationFunctionType.Relu)

                    ps_ut = ps2t.tile([128, 128], bf16, tag="ps_ut")
                    nc.tensor.transpose(ps_ut[:], u[:], ident16[:])
                    ut = p2.tile([128, 128], bf16, tag="ut")
                    nc.scalar.copy(ut[:], ps_ut[:])

                    ps_o = ps2.tile([128, 128], f32, tag="ps_o")
                    nc.tensor.matmul(ps_o[:], ut[:], uw216[:], start=True, stop=False)
                    nc.tensor.matmul(ps_o[:], ones_row[:], bu2[:], start=False, stop=False)
                    nc.tensor.matmul(ps_o[:], ident[:], x_own[:, b, :], start=False, stop=True)
                    nc.scalar.activation(ostage[:, b, :], ps_o[:],
                                         mybir.ActivationFunctionType.Relu)

                # software pipeline: gather 2 ahead, add/relu 1 ahead
                gs = {0: emit_gather(0)}
                if BLOCKS > 1:
                    gs[1] = emit_gather(1)
                emit_addrelu(0, gs[0])
                for b in range(BLOCKS):
                    if b + 2 < BLOCKS:
                        gs[b + 2] = emit_gather(b + 2)
                    if b + 1 < BLOCKS:
                        emit_addrelu(b + 1, gs[b + 1])
                    emit_finish(b, gs.pop(b))
                    if b % 5 == 4:
                        nc.sync.dma_start(out_v[:, b - 4:b + 1, :],
                                          ostage[:, b - 4:b + 1, :])


    nc.compile()
    nc.m = get_hw_module(nc.m)
    return nc


def get_nc():
    if "nc" not in _CACHE:
        _CACHE["nc"] = _build_nc()
    return _CACHE["nc"]


def make_in_maps(atom_features, nbr_indices,
                 msg_W1, msg_b1, msg_W2, msg_b2,
                 upd_W1, upd_b1, upd_W2, upd_b2):
    atom_features = np.ascontiguousarray(np.asarray(atom_features, dtype=np.float32))
    nbr = np.asarray(nbr_indices)

    atoms = np.zeros((N_PAD, D), dtype=np.float32)
    atoms[:N_ATOMS] = atom_features
    import ml_dtypes
    atoms16 = atoms.astype(ml_dtypes.bfloat16)

    idx = np.zeros((N_PAD, M), dtype=np.int16)
    idx[:N_ATOMS] = nbr.astype(np.int16)
    # per core/block: logical order j = m*128 + p; wrapped [16, 256] then
    # replicated to 128 partitions: unwrapped[j] = tile[j % 16, j // 16]
    idx = idx.reshape(N_CORES, BLOCKS, 128, M)
    idx = idx.transpose(0, 1, 3, 2)                 # [core, blk, m, p] -> L[j]
    idx = idx.reshape(N_CORES, BLOCKS * M * 128 // 16, 16)
    idx = idx.transpose(0, 2, 1)                    # [core, 16, 6400]
    idx16 = np.tile(idx, (1, 8, 1))                 # [core, 128, 6400]
    idx16 = np.ascontiguousarray(idx16)

    w = {
        "w1a": np.ascontiguousarray(np.asarray(msg_W1[:D], dtype=np.float32)),
        "w1b": np.ascontiguousarray(np.asarray(msg_W1[D:], dtype=np.float32)),
        "b1": np.asarray(msg_b1, dtype=np.float32).reshape(1, D),
        "w2": np.ascontiguousarray(np.asarray(msg_W2, dtype=np.float32)),
        "b2c": np.asarray(msg_b2, dtype=np.float32).reshape(D, 1),
        "u1a": np.ascontiguousarray(np.asarray(upd_W1[:D], dtype=np.float32)),
        "u1b": np.ascontiguousarray(np.asarray(upd_W1[D:], dtype=np.float32)),
        "bu1": np.asarray(upd_b1, dtype=np.float32).reshape(1, D),
        "uw2": np.ascontiguousarray(np.asarray(upd_W2, dtype=np.float32)),
        "bu2": np.asarray(upd_b2, dtype=np.float32).reshape(1, D),
    }

    in_maps = []
    for c in range(N_CORES):
        m = {
            "atoms16": atoms16,
            "own_x": atoms[c * OWN:(c + 1) * OWN],
            "own_x16": atoms16[c * OWN:(c + 1) * OWN],
            "idx16": idx16[c],
        }
        m.update(w)
        in_maps.append(m)
    return in_maps


def kernel(atom_features, nbr_features, nbr_indices,
           msg_W1, msg_b1, msg_W2, msg_b2,
           upd_W1, upd_b1, upd_W2, upd_b2):
    global last_results
    from concourse.bass_utils import run_bass_kernel_spmd

    nc = get_nc()
    in_maps = make_in_maps(atom_features, nbr_indices,
                           msg_W1, msg_b1, msg_W2, msg_b2,
                           upd_W1, upd_b1, upd_W2, upd_b2)
    res = run_bass_kernel_spmd(nc, in_maps, core_ids=list(range(N_CORES)))
    last_results = res
    out = np.concatenate([res.results[c]["out"] for c in range(N_CORES)], axis=0)
    return out[:N_ATOMS]
